# revision 18
# baseline (speedup 1.0000x reference)
"""FFM (field-aware factorization machine) forward kernel for 8 TRN2 NeuronCores.

y[b] = x[b] @ w_lin + b_lin + sum_{i<j} Wu[i,j] x[b,i] x[b,j]
with Wu = triu(Wmat, 1), Wmat[i,j] = <v[i, field[j]], v[j, field[i]]>.

Strategy (v2):
  - Host: build Wmat from (v, field_idx)  [tiny: 256x256x8], symmetrize
    S = (Wu + Wu^T)/2, eigendecompose S = Q diag(lam) Q^T.  Then
    x^T Wu x = sum_n lam_n (x . q_n)^2 = sum_n sign_n * (sqrt|lam_n| x . q_n)^2.
    sqrt|lam| is folded into Q's columns on the host; eigendirections are
    paired into 128 same-sign (slot p, slot p+128) pairs so the two squared
    z-groups can be pre-summed on-device before a single sign-weighted
    partition-reduce matmul (5 PE passes/sample instead of 6).
  - All device inputs are fp16: halves HBM traffic vs fp32 (the kernel is
    jointly DMA/PE-bound), matmuls run at full 1 col/cycle rate.
  - Device (data-parallel over batch, 8 cores), per 512-sample chunk:
      PE:  4 z-matmuls  pz0/pz1[128,512] = Q'^T X^T   (fp16 in, fp32 psum)
      ACT: zs0 = Square(pz0)              -> fp16 SBUF
      DVE: zs1 = pz1 * pz1                -> fp16 SBUF (direct from PSUM)
      DVE: s   = zs0 + zs1                   fp16, 4x-rate mode
      PE:  py[16,512] (+)= sgn_mask^T @ s    one reduce matmul per chunk;
           chunk c lands on psum partition row c%16, so 16 chunks copy out
           as one [16,512] ACT copy.
  - Host feeds x pre-transposed ([256, B/8] per core, fp16) so the
    contraction dim lands on SBUF partitions with zero on-device transposes.
"""

import numpy as np

_B, _N = 65536, 256
_NCORES = 8
_BS = _B // _NCORES  # 8192 batch rows per core
_FCH = 512           # matmul moving free-dim chunk (1 PSUM bank of fp32)
_DCH = 2048          # DMA chunk columns (512 KiB per half-slab in fp16)
_NCH = _BS // _FCH   # 16 chunks per core

_compiled_nc = {}


_COL_PLAN = (1024, 1536, 1536, 2048, 2048)


def _build_nc(reps=1, col_plan=_COL_PLAN, n_warm=5, xin_bufs=3,
              reduce_lag=3):
    from concourse import bacc, mybir, tile

    f32 = mybir.dt.float32
    f16 = mybir.dt.float16
    Act = mybir.ActivationFunctionType

    nc = bacc.Bacc("TRN2", target_bir_lowering=False, debug=False)

    xt = nc.dram_tensor("xt", [_N, _BS], f16, kind="ExternalInput").ap()
    # packed constants: cols 0:256 = Q' rows 0:128, 256:512 = Q' rows 128:256,
    # 512:528 = masked sign table sgn[p, m*4+j] = sign[p]*(j==m), m = chunk%4
    qc = nc.dram_tensor("qc", [128, 528], f16, kind="ExternalInput").ap()
    # y_dram[m, g*512 + f] = y of chunk g*4+m (see _unscramble)
    y = nc.dram_tensor("y", [4, (_NCH // 4) * _FCH], f32,
                       kind="ExternalOutput").ap()

    assert sum(col_plan) == _BS
    assert all(c % _FCH == 0 for c in col_plan), col_plan
    starts = [sum(col_plan[:i]) for i in range(len(col_plan))]
    xin_cols = max(col_plan)

    chunks_per_rep = _NCH
    # chunk -> (slab index, col offset within slab)
    chunk_slab = []
    for d, (c0, cols) in enumerate(zip(starts, col_plan)):
        for k in range(cols // _FCH):
            chunk_slab.append((d, k))

    with tile.TileContext(nc) as tc:
        with (
            tc.tile_pool(name="const", bufs=1) as cpool,
            tc.tile_pool(name="xin", bufs=xin_bufs) as xpool,
            tc.tile_pool(name="zsq", bufs=3) as zpool,
            tc.tile_pool(name="yout", bufs=1) as ypool,
            tc.tile_pool(name="pz", bufs=2, space="PSUM") as pzpool,
            tc.tile_pool(name="pzs", bufs=2, space="PSUM") as pzspool,
            tc.tile_pool(name="py", bufs=2, space="PSUM") as pypool,
        ):
            # ACT spline-table prewarm: a tiny Square early so the one-time
            # table load overlaps the first x DMA instead of stalling chunk 0.
            warm = cpool.tile([128, 8], f32)
            nc.vector.memset(warm[:], 0.0)
            warm2 = cpool.tile([128, 8], f32)
            nc.scalar.activation(warm2[:], warm[:], Act.Square)

            qc_sb = cpool.tile([128, 528], f16)
            nc.sync.dma_start(qc_sb[:], qc[:, :])
            q0 = qc_sb[:, 0:256]
            q1 = qc_sb[:, 256:512]
            sgn_sb = qc_sb[:, 512:528]

            # PE clock-ramp warmup: garbage matmuls on zeroed data so the PE
            # reaches full rate (and HW HAM un-throttles) while the first x
            # slab is still in flight.  Uses the py pool ring; the real
            # accumulation groups later overwrite it (start=True).
            wx = cpool.tile([128, _FCH], f16)
            nc.vector.memset(wx[:], 0.0)
            pw = pzspool.tile([128, 2 * _FCH], f32, tag="pz0s")
            for _ in range(n_warm):
                nc.tensor.matmul(pw[:, 0:_FCH], wx[:, 0:128], wx[:],
                                 start=True, stop=True)

            y_sb = ypool.tile([4, (_NCH // 4) * _FCH], f32)

            state = {"py_t": None}
            last_c = reps * chunks_per_rep - 1

            def emit_reduce(item):
                c, rhs_list = item
                m = c % 4
                if m == 0:
                    state["py_t"] = pypool.tile([4, _FCH], f32, tag="py",
                                                name="py_t")
                py_t = state["py_t"]
                # lhsT col j = sign * (j == m): chunk lands on psum row m,
                # rows != m accumulate zeros.
                for i, rhs in enumerate(rhs_list):
                    nc.tensor.matmul(py_t[:],
                                     sgn_sb[:, m * 4:(m + 1) * 4], rhs,
                                     start=(m == 0 and i == 0),
                                     stop=(m == 3 and i == len(rhs_list) - 1))
                if m == 3:
                    g = (c // 4) % 4
                    nc.scalar.copy(y_sb[:, g * _FCH:(g + 1) * _FCH], py_t[:])

            pending = []
            x_tiles = {}
            pair = {}
            for _rep in range(reps):
              for ci, (d, k) in enumerate(chunk_slab):
                c = _rep * chunks_per_rep + ci
                if k == 0:
                    c0, cols = starts[d], col_plan[d]
                    x0 = xpool.tile([128, xin_cols], f16, tag="x0")
                    x1 = xpool.tile([128, xin_cols], f16, tag="x1")
                    nc.sync.dma_start(x0[:, 0:cols], xt[0:128, c0:c0 + cols])
                    nc.sync.dma_start(x1[:, 0:cols],
                                      xt[128:256, c0:c0 + cols])
                    x_tiles[d] = (x0, x1)
                x0, x1 = x_tiles[d]
                r0 = x0[:, k * _FCH:(k + 1) * _FCH]
                r1 = x1[:, k * _FCH:(k + 1) * _FCH]
                j = c % 2
                in_tail = c >= last_c - 1
                if j == 0 or in_tail:
                    # two-bank super tile: a chunk pair shares it so ACT/DVE
                    # batch their elementwise work over 1024 columns.  The
                    # tail chunks each get their own (half-used) super so
                    # their elementwise work pipelines per-chunk.
                    pair["pz0s"] = pzspool.tile([128, 2 * _FCH], f32,
                                                tag="pz0s", name="pz0s")
                    pair["t1s"] = zpool.tile([128, 2 * _FCH], f16, tag="t1s",
                                             name="t1s")
                jj = 0 if in_tail else j
                pz0 = pair["pz0s"][:, jj * _FCH:(jj + 1) * _FCH]
                pz1 = pzpool.tile([128, _FCH], f32, tag="pz1")
                # z'^T[n, b] = sum_i Q'[i, n] * xT[i, b], slot-chunk 0 / 1
                nc.tensor.matmul(pz0, q0[:, 0:128], r0,
                                 start=True, stop=False)
                nc.tensor.matmul(pz0, q1[:, 0:128], r1,
                                 start=False, stop=True)
                nc.tensor.matmul(pz1[:], q0[:, 128:256], r0,
                                 start=True, stop=False)
                nc.tensor.matmul(pz1[:], q1[:, 128:256], r1,
                                 start=False, stop=True)
                # Reduce a lagged chunk while this one's squares cook:
                # keeps PE fed and gives ACT/DVE latency slack.
                if len(pending) >= reduce_lag:
                    emit_reduce(pending.pop(0))
                t1 = pair["t1s"][:, jj * _FCH:(jj + 1) * _FCH]
                # DVE can't dual-read PSUM: copy (with fp16 cast) now, square
                # at 2x rate in SBUF once the pair completes.  Copies
                # alternate ACT/DVE to balance engine load.
                if j == 0 and not in_tail:
                    nc.scalar.copy(t1, pz1[:])
                else:
                    nc.vector.tensor_copy(t1, pz1[:])
                if in_tail:
                    # per-chunk drain path: short serial chain, no batching;
                    # the last chunk skips the pre-sum entirely (two direct
                    # reduce matmuls) to shorten the close-out.
                    zs0 = zpool.tile([128, _FCH], f16, tag="zs0t")
                    zs1 = zpool.tile([128, _FCH], f16, tag="zs1t")
                    nc.scalar.activation(zs0[:], pz0, Act.Square)
                    nc.vector.tensor_mul(zs1[:], t1, t1)
                    if c == last_c:
                        pending.append((c, [zs0[:], zs1[:]]))
                    else:
                        s_t = zpool.tile([128, _FCH], f16, tag="st")
                        nc.vector.tensor_add(s_t[:], zs0[:], zs1[:])
                        pending.append((c, [s_t[:]]))
                elif j == 1:
                    zs0s = zpool.tile([128, 2 * _FCH], f16, tag="zs0s")
                    zs1s = zpool.tile([128, 2 * _FCH], f16, tag="zs1s")
                    ss = zpool.tile([128, 2 * _FCH], f16, tag="ss")
                    nc.scalar.activation(zs0s[:], pair["pz0s"][:], Act.Square)
                    t1s = pair["t1s"]
                    nc.vector.tensor_mul(zs1s[:], t1s[:], t1s[:])
                    nc.vector.tensor_add(ss[:], zs0s[:], zs1s[:])
                    pending.append((c - 1, [ss[:, 0:_FCH]]))
                    pending.append((c, [ss[:, _FCH:2 * _FCH]]))
            for item in pending:
                emit_reduce(item)

            nc.sync.dma_start(y[:, :], y_sb[:])

    nc.compile()
    return nc


def _get_nc(reps=1, **kw):
    key = (reps,) + tuple(sorted(kw.items()))
    if key not in _compiled_nc:
        _compiled_nc[key] = _build_nc(reps, **kw)
    return _compiled_nc[key]


def _host_prep(x, w_lin, b_lin, v, field_idx):
    """Host-side tiny-param preprocessing + sharding. Returns (in_maps, lin)."""
    x = np.asarray(x, dtype=np.float32)
    w_lin = np.asarray(w_lin, dtype=np.float32)
    b_lin = np.asarray(b_lin, dtype=np.float32)
    v = np.asarray(v, dtype=np.float64)
    field_idx = np.asarray(field_idx, dtype=np.int64)

    # Wmat[i, j] = <v[i, field[j]], v[j, field[i]]>
    A = v[:, field_idx, :]                       # [N, N, K]
    Wmat = np.einsum('ijk,jik->ij', A, A)        # [N, N]
    Wu = np.triu(Wmat, 1)
    S = (Wu + Wu.T) * 0.5
    lam, Q = np.linalg.eigh(S)                   # S = Q diag(lam) Q^T, lam asc.

    # Pair eigendirections (2p, 2p+1) -> slots (p, p+128).  lam is sorted
    # ascending so only the pair straddling the sign boundary can mix signs;
    # if the negative count is odd, zero out the smaller-|lam| member of that
    # pair (error ~lam_min ~ 0.1% of output norm).
    n_neg = int(np.sum(lam < 0))
    if n_neg % 2 == 1:
        a, b = n_neg - 1, n_neg  # last negative, first positive
        drop = a if abs(lam[a]) <= abs(lam[b]) else b
        lam[drop] = 0.0
        Q[:, drop] = 0.0
    qs = Q * np.sqrt(np.abs(lam))[None, :]       # scaled eigvecs [N, N]
    # slot p <- dir 2p ; slot p+128 <- dir 2p+1
    q_arranged = np.empty((_N, _N), dtype=np.float64)
    q_arranged[:, 0:128] = qs[:, 0::2]
    q_arranged[:, 128:256] = qs[:, 1::2]
    sign = np.where(lam[0::2] + lam[1::2] >= 0, 1.0, -1.0)  # [128] pair sign

    q16 = q_arranged.astype(np.float16)
    # masked sign table [128, 16]: col m*4 + j = sign[p] * (j == m), m = c%4
    sgn_tbl = np.zeros((128, 4, 4), dtype=np.float16)
    for m in range(4):
        sgn_tbl[:, m, m] = sign.astype(np.float16)
    sgn16 = sgn_tbl.reshape(128, 16)
    # packed constants [128, 528]: Q' rows 0:128 | Q' rows 128:256 | sign table
    qc16 = np.concatenate([q16[0:128, :], q16[128:256, :], sgn16], axis=1)
    qc16 = np.ascontiguousarray(qc16)

    # x transposed + sharded along batch, cast to fp16
    xts = x.reshape(_NCORES, _BS, _N).transpose(0, 2, 1)  # [8, N, BS]
    xts = np.ascontiguousarray(xts).astype(np.float16)

    in_maps = [
        {"xt": xts[i], "qc": qc16} for i in range(_NCORES)
    ]
    lin = x @ w_lin + b_lin[0]                   # linear part on host (0.4% of FLOPs)
    return in_maps, lin


def _unscramble(y_core):
    """[4, 2048] device layout -> [8192] batch order.

    Chunk c (samples c*512..c*512+511) lands on row c%4, cols (c//4)*512.."""
    return y_core.reshape(4, _NCH // 4, _FCH).transpose(1, 0, 2).reshape(_BS)


def _run_device(in_maps, trace=False, reps=1):
    from concourse.bass_utils import run_bass_kernel_spmd

    nc = _get_nc(reps)
    res = run_bass_kernel_spmd(
        nc, in_maps, core_ids=list(range(_NCORES)), trace=trace
    )
    yq = np.concatenate(
        [_unscramble(res.results[i]["y"]) for i in range(_NCORES)]
    )
    return yq, res


def kernel(x, w_lin, b_lin, v, field_idx):
    in_maps, lin = _host_prep(x, w_lin, b_lin, v, field_idx)
    yq, _ = _run_device(in_maps, trace=False)
    return (lin + yq).astype(np.float32)[:, None]
